# revision 85
# baseline (speedup 1.0000x reference)
"""CapsuleLayer dynamic-routing kernel for 8 Trainium2 NeuronCores.

Problem: u_hat[b,i,j,e] = einsum('bid,ijde->bije', x, W) with
B=64, I=2304, D=8, J=32, E=16, followed by NUM_ROUTING=3 softmax
routing iterations.  Output V = squash(S_2) with shape [B, J, E].

Strategy (data-parallel over batch, 8 b per core):
 - Host pre-lays W into fp16 tiles [G=144, 128, 512] with partition
   p = (i_loc*8 + d) and free f = (e*32 + j); W streams through SBUF once.
 - Phase 1: a block-diagonal lhsT (built on GPSIMD with one batched mask
   multiply per 8-group chunk) makes ONE matmul produce u_hat for
   16 i's x 8 b x (all j,e) per group; a second matmul per group
   accumulates J*S_0 in PSUM (the 1/J lands in the squash scale).
   PSUM->SBUF evacuation is a pure dtype-converting copy alternating
   DVE/ACT.  A few dummy matmuls during the first W DMA ramp the PE
   p-state to 2.4 GHz before real work arrives.
 - u_hat stays resident in SBUF as fp16 [128=(il,b), G*512=(g,(e,j))].
 - Routing exploits b-update linearity: logits(it2) = b0 + u.(V0+V1),
   so no logit tensor is carried between iterations.
 - Per iteration the agreement reduction over e runs on the TENSOR
   engine: P = u*V_rep is written in (e,g,j) order and 16 identity-lhsT
   matmuls accumulate the e-slices into a PSUM logit tile (replacing a
   DVE reduction tree).  exp() runs batched on ACT straight from PSUM
   in bf16 (f32 range - cannot overflow), written (j,g)-transposed so
   the softmax denominator is 32 more tiny accumulating PE matmuls.
 - The softmax scale 1/Z never touches the data tensors: a second
   bf16 exp (f32 exponent range - cannot overflow) feeds T = u*exp(a)
   and the 1/Z rides the S-matmul lhsT as a tiny bf16 indicator*recip
   tile, deleting the c = exp*recip multiply from the DVE entirely.
 - DVE keeps only: P = u*V_rep, T = u*exp (both 2-byte 2x
   TensorTensor), recip, and the squash tail.  Every third P-multiply
   runs whole on GPSIMD, emitted P_LOOKAHEAD macros early so the
   in-order PE stream (which must stay dense to hold its p-state)
   never waits.
"""

import sys

import ml_dtypes
import numpy as np

sys.path.insert(0, "/opt/trn_rl_repo")

B, I, D, J, E = 64, 2304, 8, 32, 16
NC_CORES = 8
BS = B // NC_CORES          # 8 batch elements per core
IL = 16                     # i's per group
G = I // IL                 # 144 groups
F = J * E                   # 512 free elements per group
GB = 8                      # groups per batched macro-op
GJ = GB * J                 # free size of one macro logit tile
P_BUFS = 4                  # product-tile buffering (>= P_LOOKAHEAD + 2)
W_BUFS = 4                  # W-stream buffering
SM_BUFS = 5                 # softmax small-tile buffering
A_BUFS = 3                  # logit PSUM buffering
SPS_BUFS = 1                # S-accumulator PSUM buffering
KEEPALIVE = 16              # idle matmuls across iteration boundaries so the
                            # PE p-state never resets during squash/vrep
GD = 8                      # groups per W DMA batch (phase-1 fill granularity)
NUM_ROUTING = 3

# engine-balance knobs
POOL_P_EVERY = 3            # every Nth P-multiply goes to GPSIMD (0=off)
POOL_P_NUM = 0              # Bresenham offload: NUM of every DEN macros on
POOL_P_DEN = 5              # GPSIMD (overrides POOL_P_EVERY when NUM>0)
T_DEFER = 1                 # macros to defer GPSIMD-T S-matmuls by
SQ_BUFS = 1                 # squash scratch buffering
RC_IN_LT = True             # fold 1/Z into a bf16 S-matmul lhsT (indicator*rc)
                            # so T = u*exp(a) directly in bf16 (f32-range exp
                            # cannot overflow bf16); deletes the cc multiply
POOL_T_EVERY = 0            # every Nth T-multiply goes to GPSIMD (0=off)
CC_VIA_ACT = False          # c = exp(a + ln(1/Z)) on ACT (SLOW: table reloads)
CC_ON_POOL = False          # c = exp * recip multiply on GPSIMD
P_POOL_E = 0                # e-slices of each P-multiply done on GPSIMD (0-16)
T_BUFS = 2                  # T-tile buffering
T_POOL_G = 0                # g-slices of each T-multiply done on GPSIMD (0-6);
                            # those groups' S-matmuls run one macro late
P_LOOKAHEAD = 2             # emit P-mult of macro m+LOOKAHEAD before stage_r(m)
B_LAG = 2                   # stage_b runs B_LAG macros behind stage_r
Z_BUFS = 2                  # Z-denominator PSUM buffering
Z_ON_POOL = True            # softmax denominator tree on GPSIMD
Z_ON_PE = True              # softmax denominator via accumulating PE matmuls
                            # over a (j,g)-transposed bf16 exp (overrides
                            # Z_ON_POOL; frees GPSIMD for P/T offload)
LHST_ON_POOL = True         # phase-1 block-diagonal lhsT build on GPSIMD
# PSUM->SBUF evac engine pattern, cycled per 2-group chunk
EVAC_PATTERN = "ad"         # d=DVE, a=ACT (GPSIMD cannot access PSUM)
WARMUP_MM = 16              # dummy matmuls during the first W DMA to ramp the
                            # PE p-state (2.4 GHz needs ~3 us continuous busy)
FP8_PH1 = False             # phase 1 in fp8e4 (halves the W DMA volume)
FP8_DR = False              # use DoubleRow perf mode on the fp8 matmuls
                            # (halves PE time in the cost model, but real
                            # neuronxcc rejects it: s3_lw_dual_fp8_restrictions)

_CACHE = {}


N_PASSES = 2                # routing passes (2 = full; lower for profiling)


def _build_program(n_groups, nonzero_b0):
    import concourse.bass as bass
    import concourse.mybir as mybir
    import concourse.tile as tile
    from concourse import bacc

    fp16 = mybir.dt.float16
    bf16 = mybir.dt.bfloat16
    fp8 = mybir.dt.float8e4
    f32 = mybir.dt.float32

    nc = bacc.Bacc("TRN2", target_bir_lowering=False, debug=False)

    # register the squash-epsilon constant for activation bias
    eps_t = nc.alloc_sbuf_tensor("const-f32-eps", [128, 1], f32)
    nc.gpsimd.memset(eps_t.ap(), 1e-7)
    nc.const_aps.aps[(f32, 1e-7)] = eps_t.ap()
    nc.all_engine_barrier()

    g_ = n_groups
    wdt = fp8 if FP8_PH1 else fp16
    wdt_sb = wdt
    gw = g_ + 1 if FP8_PH1 else g_
    wp = nc.dram_tensor("wp", [gw, 128, F], wdt, kind="ExternalInput").ap()
    xs = nc.dram_tensor("xs", [128, g_, BS], wdt, kind="ExternalInput").ap()
    msk = nc.dram_tensor("msk", [128, 128], fp16, kind="ExternalInput").ap()
    ind = nc.dram_tensor("ind", [128, BS], fp16, kind="ExternalInput").ap()
    vind = nc.dram_tensor("vind", [BS, 128], fp16, kind="ExternalInput").ap()
    idn = nc.dram_tensor("idn", [128, 128], fp16, kind="ExternalInput").ap()
    idb = nc.dram_tensor("idb", [128, 128], bf16, kind="ExternalInput").ap()
    if nonzero_b0:
        wp0 = nc.dram_tensor("wp0", [gw, 128, F], wdt, kind="ExternalInput").ap()
        b0p = nc.dram_tensor("b0p", [128, g_ * J], fp16, kind="ExternalInput").ap()
    v_out = nc.dram_tensor("v_out", [BS, F], f32, kind="ExternalOutput").ap()

    from contextlib import ExitStack

    byp = mybir.AluOpType.bypass
    mul = mybir.AluOpType.mult
    add = mybir.AluOpType.add

    with tile.TileContext(nc) as tc:
        with ExitStack() as ctx:
            ent = ctx.enter_context
            uhat_pool = ent(tc.tile_pool(name="uhat", bufs=1))
            cst_pool = ent(tc.tile_pool(name="cst", bufs=1))
            sm_pool = ent(tc.tile_pool(name="sm", bufs=SM_BUFS))
            vrep_pool = ent(tc.tile_pool(name="vrep", bufs=1))
            sq_pool = ent(tc.tile_pool(name="sq", bufs=SQ_BUFS))
            s0_stack = ExitStack()
            s0_psum = s0_stack.enter_context(
                tc.tile_pool(name="s0ps", bufs=1, space="PSUM")
            )
            phase1 = ExitStack()
            xs_pool = phase1.enter_context(tc.tile_pool(name="xsp", bufs=1))
            w_pool = phase1.enter_context(tc.tile_pool(name="wstream", bufs=W_BUFS))
            l_pool = phase1.enter_context(tc.tile_pool(name="lstream", bufs=3))
            mm_psum = phase1.enter_context(
                tc.tile_pool(name="mmps", bufs=3, space="PSUM")
            )
            # ---- persistent SBUF tensors ----
            uhat = uhat_pool.tile([128, g_ * F], fp16)       # (g,(e,j)) per part
            uv = uhat[:].rearrange("p (g f) -> p g f", g=g_)
            xs_sb = xs_pool.tile([128, g_ * BS], wdt)
            xsv = xs_sb[:].rearrange("p (g b) -> p g b", g=g_)
            ind_sb = cst_pool.tile([128, BS], fp16)
            vind_sb = cst_pool.tile([BS, 128], fp16)
            msk_sb = cst_pool.tile([128, 128], fp16)
            idn_sb = cst_pool.tile([128, 128], fp16)
            idb_sb = cst_pool.tile([128, 128], bf16)
            if nonzero_b0:
                b0_sb = cst_pool.tile([128, g_ * J], fp16)
                b0v = b0_sb[:].rearrange("p (g j) -> p g j", g=g_)

            nc.sync.dma_start(xs_sb[:], xs.rearrange("p g b -> p (g b)"))
            nc.sync.dma_start(ind_sb[:], ind)
            nc.sync.dma_start(vind_sb[:], vind)
            nc.sync.dma_start(msk_sb[:], msk)
            nc.sync.dma_start(idn_sb[:], idn)
            nc.sync.dma_start(idb_sb[:], idb)
            if nonzero_b0:
                nc.sync.dma_start(b0_sb[:], b0p)

            # ---- phase 1: u_hat + J*S0 ----
            # W DMA in batches of GD groups.  One batched mask multiply
            # builds the block-diagonal lhsT for all GD groups; u_hat
            # lands in PSUM unscaled and is evacuated by pure copies
            # round-robined over DVE/ACT/Pool.
            s0 = s0_psum.tile([BS, F], f32)
            assert g_ % GD == 0
            evac_idx = 0
            if WARMUP_MM:
                # ramp the PE p-state while the first W tile is in flight
                wu = s0_psum.tile([128, 128], f32, tag="warm")
                for _ in range(WARMUP_MM):
                    nc.tensor.matmul(
                        wu[:], lhsT=idn_sb[:], rhs=idn_sb[:],
                        start=True, stop=True,
                    )
            leng = nc.gpsimd if LHST_ON_POOL else nc.vector
            if FP8_PH1 and FP8_DR:
                # fp8e4 W/x with DoubleRow matmuls: each group's lhsT is a
                # [128, 2, 128] pair (block-diagonal x, zero plane); the rhs
                # pair is (W_g, W_{g+1}) where the second plane multiplies
                # zeros.  wp is host-padded by one zero group so the pair
                # addressing never leaves the tile.  PE cost halves.
                la_pool = phase1.enter_context(
                    tc.tile_pool(name="ltarena", bufs=1)
                )
                lt_arenas = []
                for i in range(W_BUFS):
                    lt_arena = la_pool.tile([128, GD * 2 * 128], fp8,
                                            tag="lt%d" % i)
                    nc.gpsimd.memset(lt_arena[:], 0.0)
                    lt_arenas.append(lt_arena)
                dr = mybir.MatmulPerfMode.DoubleRow
            for gd in range(g_ // GD):
                g0 = gd * GD
                if FP8_PH1:
                    wt = w_pool.tile([128, (GD + 1) * F], fp8)
                    wtv = wt[:].rearrange("p (g f) -> p g f", g=GD + 1)
                    nc.sync.dma_start(
                        wtv, wp[g0:g0 + GD + 1].rearrange("g p f -> p g f")
                    )
                else:
                    wt = w_pool.tile([128, GD * F], fp16)
                    wtv = wt[:].rearrange("p (g f) -> p g f", g=GD)
                    nc.sync.dma_start(
                        wtv, wp[g0:g0 + GD].rearrange("g p f -> p g f")
                    )
                if nonzero_b0:
                    nw0 = GD + 1 if FP8_PH1 else GD
                    w0t = w_pool.tile([128, nw0 * F], fp8 if FP8_PH1 else fp16,
                                      tag="w0t")
                    w0tv = w0t[:].rearrange("p (g f) -> p g f", g=nw0)
                    nc.sync.dma_start(
                        w0tv, wp0[g0:g0 + nw0].rearrange("g p f -> p g f")
                    )
                # batched block-diagonal lhsT for GD groups: one engine op
                xsb = xsv[:, g0:g0 + GD][:, :, None, :].broadcast_to(
                    [128, GD, IL, BS]
                )
                mskb = msk_sb[:].rearrange("p (i b) -> p i b", i=IL)[
                    :, None, :, :
                ].broadcast_to([128, GD, IL, BS])
                if FP8_PH1 and FP8_DR:
                    lt = lt_arenas[gd % W_BUFS]
                    ltv = lt[:].rearrange(
                        "p (g h i b) -> p g h i b", g=GD, h=2, i=IL
                    )[:, :, 0]
                    lt2 = lt[:].rearrange("p (g hm) -> p g hm", g=GD)
                else:
                    lt = l_pool.tile([128, GD * 128], wdt_sb)
                    ltv = lt[:].rearrange("p (g i b) -> p g i b", g=GD, i=IL)
                leng.tensor_tensor(ltv, xsb, mskb, op=mul)
                for h in range(GD // 2):
                    ps = mm_psum.tile([128, 2 * F], f32)
                    for k in range(2):
                        g = g0 + h * 2 + k
                        gk = h * 2 + k
                        if FP8_PH1 and FP8_DR:
                            nc.tensor.matmul(
                                ps[:, k * F:(k + 1) * F],
                                lhsT=lt2[:, gk].rearrange(
                                    "p (h m) -> p h m", h=2
                                ),
                                rhs=wt[:, gk * F:(gk + 2) * F].rearrange(
                                    "p (h f) -> p h f", h=2
                                ),
                                start=True, stop=True, perf_mode=dr,
                            )
                            if k == 0:
                                s0w = w0t if nonzero_b0 else wt
                                nc.tensor.matmul(
                                    s0[:],
                                    lhsT=xsv[:, g:g + 2],
                                    rhs=s0w[:, gk * F:(gk + 2) * F].rearrange(
                                        "p (h f) -> p h f", h=2
                                    ),
                                    start=(g == 0), stop=(g == g_ - 2),
                                    perf_mode=dr,
                                )
                        else:
                            nc.tensor.matmul(
                                ps[:, k * F:(k + 1) * F],
                                lhsT=lt[:, gk * 128:(gk + 1) * 128],
                                rhs=wtv[:, gk], start=True, stop=True,
                            )
                            s0_rhs = (
                                w0tv[:, gk] if nonzero_b0 else wtv[:, gk]
                            )
                            nc.tensor.matmul(
                                s0[:], lhsT=xsv[:, g], rhs=s0_rhs,
                                start=(g == 0), stop=(g == g_ - 1),
                            )
                    gg = g0 + h * 2
                    dst = uhat[:, gg * F:(gg + 2) * F]
                    e = EVAC_PATTERN[evac_idx % len(EVAC_PATTERN)]
                    evac_idx += 1
                    if e == "a":
                        nc.scalar.activation(
                            dst, ps[:], mybir.ActivationFunctionType.Copy
                        )
                    elif e == "p":
                        nc.gpsimd.tensor_copy(dst, ps[:])
                    else:
                        nc.vector.tensor_copy(dst, ps[:])

            # free the phase-1 streaming pools; routing pools reuse the space
            phase1.close()
            p_pool = ent(tc.tile_pool(name="ptree", bufs=P_BUFS))
            t_pool = ent(tc.tile_pool(name="ttile", bufs=T_BUFS))
            routing_psum = {}

            def open_routing_psum():
                # deferred until the S0 PSUM bank is released
                routing_psum["s"] = ent(tc.tile_pool(name="sps", bufs=SPS_BUFS, space="PSUM"))
                routing_psum["a"] = ent(
                    tc.tile_pool(name="aps", bufs=A_BUFS, space="PSUM")
                )
                routing_psum["z"] = ent(tc.tile_pool(name="zps", bufs=Z_BUFS, space="PSUM"))
                routing_psum["vr"] = ent(
                    tc.tile_pool(name="vrps", bufs=1, space="PSUM")
                )
                routing_psum["w"] = ent(
                    tc.tile_pool(name="wps", bufs=1, space="PSUM")
                )

            def pe_keepalive():
                if not KEEPALIVE:
                    return
                ka = routing_psum["w"].tile([128, 128], f32, tag="warm")
                for _ in range(KEEPALIVE):
                    nc.tensor.matmul(
                        ka[:], lhsT=idn_sb[:], rhs=idn_sb[:],
                        start=True, stop=True,
                    )

            def squash(s_ps, out_dt, out_pool, inv_scale):
                """s_ps: PSUM [BS, F] f32 = S/inv_scale -> V tile [BS, F]."""
                sqv = sq_pool.tile([BS, F], f32, tag="sqv")
                nc.scalar.activation(
                    sqv[:], s_ps[:], mybir.ActivationFunctionType.Square,
                    scale=float(inv_scale),
                )
                s2 = sq_pool.tile([BS, J], f32, tag="s2")
                # reduce over e (outer dim): view (j, e) with e innermost
                sq3 = sqv[:].rearrange("p (e j) -> p j e", e=E)
                nc.vector.tensor_reduce(
                    s2[:], sq3, axis=mybir.AxisListType.X, op=add
                )
                rt = sq_pool.tile([BS, J], f32, tag="rt")
                nc.scalar.activation(
                    rt[:], s2[:], mybir.ActivationFunctionType.Sqrt, bias=1e-7
                )
                den = sq_pool.tile([BS, J], f32, tag="den")
                nc.vector.scalar_tensor_tensor(
                    den[:], s2[:], 1.0, rt[:], op0=add, op1=mul
                )
                rden = sq_pool.tile([BS, J], f32, tag="rden")
                nc.vector.reciprocal(rden[:], den[:])
                sc = sq_pool.tile([BS, J], f32, tag="sc")
                nc.vector.tensor_tensor(sc[:], s2[:], rden[:], op=mul)
                # V = (S/beta) * sc (broadcast sc over e)
                vt = out_pool.tile([BS, F], out_dt, tag="vtile")
                scb = sc[:][:, None, :].broadcast_to([BS, E, J])
                nc.vector.scalar_tensor_tensor(
                    vt[:].rearrange("p (e j) -> p e j", e=E),
                    s_ps[:].rearrange("p (e j) -> p e j", e=E),
                    float(inv_scale), scb, op0=mul, op1=mul,
                )
                return vt

            def make_vrep(v_sb):
                """v_sb [BS, F] fp16 -> V replicated to 128 partitions fp16."""
                vr_ps = routing_psum["vr"].tile([128, F], f32)
                nc.tensor.matmul(
                    vr_ps[:], lhsT=vind_sb[:], rhs=v_sb[:], start=True, stop=True
                )
                vr = vrep_pool.tile([128, F], fp16, tag="vr%d" % make_vrep.n)
                make_vrep.n += 1
                nc.scalar.activation(
                    vr[:], vr_ps[:], mybir.ActivationFunctionType.Copy
                )
                return vr

            make_vrep.n = 0

            n_mac = g_ // GB
            exp_f = mybir.ActivationFunctionType.Exp
            zeng = nc.gpsimd if Z_ON_POOL else nc.vector

            def routing_pass(vr, it, s_ps):
                """One routing iteration.

                stage A: P = u*V_rep written in (e,g,j) order (DVE TT 2x,
                or GPSIMD for offloaded macros); 16 identity-lhsT matmuls
                accumulate the e-slices into a PSUM logit tile; batched
                f32 exp on ACT; j-sum tree for the denominator on GPSIMD.
                stage B: c = exp*(1/Z) (DVE), T = u*c (TT 2x), and PE
                contracts over i with the indicator lhsT into S.
                Two-stage software pipelining keeps DVE busy across the
                ACT/PE handoffs.
                """
                pending_s = []
                s_state = {"n": 0}

                def emit_s(job):
                    g, t8_, k, slhs_, kg = job
                    lh = (
                        slhs_[:, kg * BS:(kg + 1) * BS]
                        if slhs_ is not None
                        else ind_sb[:]
                    )
                    nc.tensor.matmul(
                        s_ps[:], lhsT=lh,
                        rhs=t8_[:, k * F:(k + 1) * F],
                        start=(s_state["n"] == 0), stop=(s_state["n"] == g_ - 1),
                    )
                    s_state["n"] += 1

                def stage_p(m):
                    """P = u*V_rep in (e,g,j) order, split between DVE and
                    GPSIMD by e-range (emitted P_LOOKAHEAD macros early so
                    the slower engine finishes before PE consumes it)."""
                    g0 = m * GB
                    u8 = uv[:, g0:g0 + GB]                       # [128, GB, F]
                    u8e = u8.rearrange("p g (e j) -> p e g j", e=E)
                    p8 = p_pool.tile([128, GB * F], fp16, tag="p8")
                    p8v = p8[:].rearrange("p (e g j) -> p e g j", e=E, g=GB)
                    vrb = vr[:].rearrange("p (e j) -> p e j", e=E)[
                        :, :, None, :
                    ].broadcast_to([128, E, GB, J])
                    ec = E - P_POOL_E
                    pp = None
                    if POOL_P_NUM:
                        _ps = {(i * POOL_P_DEN + POOL_P_DEN // 2) // POOL_P_NUM
                               for i in range(POOL_P_NUM)}
                        _on_pool = (m % POOL_P_DEN) in _ps
                    else:
                        _on_pool = (
                            POOL_P_EVERY
                            and m % POOL_P_EVERY == POOL_P_EVERY - 1
                        )
                    if _on_pool:
                        nc.gpsimd.tensor_tensor(p8v, u8e, vrb, op=mul)
                    elif P_POOL_E:
                        # GPSIMD half goes to its own tile so the two engines
                        # never co-write one tile (avoids WAW serialization)
                        nc.vector.tensor_tensor(
                            p8v[:, :ec], u8e[:, :ec], vrb[:, :ec], op=mul
                        )
                        pp = p_pool.tile([128, P_POOL_E * GJ], fp16, tag="pp")
                        ppv = pp[:].rearrange(
                            "p (e g j) -> p e g j", e=P_POOL_E, g=GB
                        )
                        nc.gpsimd.tensor_tensor(ppv, u8e[:, ec:], vrb[:, ec:], op=mul)
                    else:
                        nc.vector.tensor_tensor(p8v, u8e, vrb, op=mul)
                    return p8, pp

                def stage_r(m, ptile):
                    p8, pp = ptile
                    g0 = m * GB
                    u8 = uv[:, g0:g0 + GB]
                    # e-reduction on the tensor engine: 16 accumulating
                    # identity matmuls over contiguous e-slices of p8/pp
                    ec = E - P_POOL_E if pp is not None else E
                    aps = routing_psum["a"].tile([128, GJ], f32)
                    for e in range(E):
                        src = (
                            p8[:, e * GJ:(e + 1) * GJ]
                            if e < ec
                            else pp[:, (e - ec) * GJ:(e - ec + 1) * GJ]
                        )
                        nc.tensor.matmul(
                            aps[:], lhsT=idn_sb[:], rhs=src,
                            start=(e == 0),
                            stop=(e == E - 1 and not nonzero_b0),
                        )
                    if nonzero_b0:
                        nc.tensor.matmul(
                            aps[:], lhsT=idn_sb[:],
                            rhs=b0_sb[:, g0 * J:(g0 + GB) * J],
                            start=False, stop=True,
                        )
                    if Z_ON_PE:
                        # exp in bf16 (same range as f32: cannot overflow),
                        # written (j,g)-transposed so the denominator is 32
                        # accumulating identity matmuls over contiguous
                        # g-slices (emitted in stage_b to decouple PE from
                        # the ACT latency)
                        ex = sm_pool.tile([128, GJ], bf16, tag="ex")
                        ex_t = ex[:].rearrange("p (j g) -> p g j", g=GB)
                        nc.scalar.activation(
                            ex_t, aps[:].rearrange("p (g j) -> p g j", g=GB),
                            exp_f,
                        )
                        return u8, ex, None, aps
                    ex = sm_pool.tile([128, GJ], f32, tag="ex")
                    nc.scalar.activation(ex[:], aps[:], exp_f)
                    # denominator: j-tree 32->16->8->4->2->1 into scratch
                    zz = sm_pool.tile([128, GB * 16], f32, tag="zz")
                    zzv = zz[:].rearrange("p (g h) -> p g h", g=GB)
                    exv = ex[:].rearrange("p (g j) -> p g j", g=GB)
                    zeng.tensor_tensor(
                        zzv, exv[:, :, 0:16], exv[:, :, 16:32], op=add
                    )
                    zeng.tensor_tensor(
                        zzv[:, :, 0:8], zzv[:, :, 0:8], zzv[:, :, 8:16], op=add
                    )
                    zeng.tensor_tensor(
                        zzv[:, :, 0:4], zzv[:, :, 0:4], zzv[:, :, 4:8], op=add
                    )
                    zeng.tensor_tensor(
                        zzv[:, :, 0:2], zzv[:, :, 0:2], zzv[:, :, 2:4], op=add
                    )
                    zeng.tensor_tensor(
                        zzv[:, :, 0:1], zzv[:, :, 0:1], zzv[:, :, 1:2], op=add
                    )
                    return u8, ex, zz, aps

                def stage_z(ex):
                    # Z denominator matmuls, hoisted ahead of the next
                    # macro's e-reduction so PE produces zps before DVE's
                    # reciprocal needs it (otherwise rc waits through an
                    # entire extra PE stage)
                    zps = routing_psum["z"].tile([128, GB], f32)
                    for j in range(J):
                        nc.tensor.matmul(
                            zps[:], lhsT=idb_sb[:],
                            rhs=ex[:, j * GB:(j + 1) * GB],
                            start=(j == 0), stop=(j == J - 1),
                        )
                    return zps

                def stage_b(m, u8, ex, zz, aps, zps):
                    g0 = m * GB
                    rc = sm_pool.tile([128, GB], f32, tag="rc")
                    if Z_ON_PE:
                        nc.vector.reciprocal(rc[:], zps[:])
                    else:
                        nc.vector.reciprocal(
                            rc[:],
                            zz[:].rearrange("p (g h) -> p g h", g=GB)[:, :, 0],
                        )
                    if RC_IN_LT:
                        # bf16 lhsT = indicator * (1/Z): the softmax scale
                        # rides the S-matmul weights instead of a cc multiply
                        ltc = sm_pool.tile([128, GB * BS], bf16, tag="ltc")
                        ltcv = ltc[:].rearrange("p (g b) -> p g b", g=GB)
                        indb = ind_sb[:][:, None, :].broadcast_to(
                            [128, GB, BS]
                        )
                        rcb2 = rc[:][:, :, None].broadcast_to([128, GB, BS])
                        nc.vector.tensor_tensor(ltcv, indb, rcb2, op=mul)
                        # second exp in (g,j) order, bf16, feeds the T mult
                        cc = sm_pool.tile([128, GJ], bf16, tag="cc2")
                        nc.scalar.activation(cc[:], aps[:], exp_f)
                        slhs = ltc
                    elif CC_VIA_ACT:
                        cc = sm_pool.tile([128, GJ], fp16, tag="cc")
                        ccv = cc[:].rearrange("p (g j) -> p g j", g=GB)
                        slhs = None
                        # c = exp(a + ln(1/Z)): per-group exp on ACT with the
                        # log-denominator as per-partition bias; c <= 1 so the
                        # fp16 output cannot overflow.
                        lz = sm_pool.tile([128, GB], f32, tag="lz")
                        nc.scalar.activation(
                            lz[:], rc[:], mybir.ActivationFunctionType.Ln
                        )
                        apv = aps[:].rearrange("p (g j) -> p g j", g=GB)
                        for k in range(GB):
                            nc.scalar.activation(
                                ccv[:, k], apv[:, k], exp_f,
                                bias=lz[:, k:k + 1],
                            )
                    else:
                        # c = exp * (1/sumexp), broadcast rc over j; fp16 out
                        cc = sm_pool.tile([128, GJ], fp16, tag="cc")
                        ccv = cc[:].rearrange("p (g j) -> p g j", g=GB)
                        exv = (
                            ex[:].rearrange("p (j g) -> p g j", g=GB)
                            if Z_ON_PE
                            else ex[:].rearrange("p (g j) -> p g j", g=GB)
                        )
                        rcb = rc[:][:, :, None].broadcast_to([128, GB, J])
                        ceng = nc.gpsimd if CC_ON_POOL else nc.vector
                        ceng.tensor_tensor(ccv, exv, rcb, op=mul)
                        slhs = None
                    # T = u * c (broadcast c over e), (g,e,j) order so the
                    # S-matmul rhs slices stay contiguous.  The last T_POOL_G
                    # groups run on GPSIMD; their S-matmuls are deferred one
                    # macro so PE's in-order stream never waits on GPSIMD.
                    tdt = bf16 if RC_IN_LT else fp16
                    t8 = t_pool.tile([128, GB * F], tdt, tag="t8")
                    t8v = t8[:].rearrange("p (g e j) -> p g e j", g=GB, e=E)
                    u8e2 = u8.rearrange("p g (e j) -> p g e j", e=E)
                    ccb = cc[:].rearrange("p (g j) -> p g j", g=GB)[
                        :, :, None, :
                    ].broadcast_to([128, GB, E, J])
                    gc = GB - T_POOL_G
                    if POOL_T_EVERY and m % POOL_T_EVERY == POOL_T_EVERY - 1:
                        # whole-macro T on GPSIMD: defer all its S-matmuls one
                        # macro so PE never waits on the slower engine
                        nc.gpsimd.tensor_tensor(t8v, u8e2, ccb, op=mul)
                        gc = 0
                        tp = t8
                    elif T_POOL_G:
                        nc.vector.tensor_tensor(
                            t8v[:, :gc], u8e2[:, :gc], ccb[:, :gc], op=mul
                        )
                        tp = t_pool.tile([128, T_POOL_G * F], tdt, tag="tp")
                        tpv = tp[:].rearrange(
                            "p (g e j) -> p g e j", g=T_POOL_G, e=E
                        )
                        nc.gpsimd.tensor_tensor(
                            tpv, u8e2[:, gc:], ccb[:, gc:], op=mul
                        )
                    else:
                        nc.vector.tensor_tensor(t8v, u8e2, ccb, op=mul)
                        gc = GB
                        tp = t8

                    while pending_s and pending_s[0][0] <= m:
                        emit_s(pending_s.pop(0)[1])
                    for k in range(gc):
                        emit_s((g0 + k, t8, k, slhs, k))
                    for k in range(gc, GB):
                        pending_s.append(
                            (m + T_DEFER, (g0 + k, tp, k - gc, slhs, k))
                        )

                # 3-stage software pipeline: P-mults run P_LOOKAHEAD macros
                # ahead so GPSIMD-offloaded products are ready when PE's
                # in-order stream reaches their e-reduction matmuls.
                ptiles = {}
                pend = []
                for m in range(min(P_LOOKAHEAD, n_mac)):
                    ptiles[m] = stage_p(m)
                for m in range(n_mac):
                    if m + P_LOOKAHEAD < n_mac:
                        ptiles[m + P_LOOKAHEAD] = stage_p(m + P_LOOKAHEAD)
                    if pend and Z_ON_PE and pend[0][5] is None:
                        pend[0] = pend[0][:5] + (stage_z(pend[0][2]),)
                    pend.append((m, *stage_r(m, ptiles.pop(m)), None))
                    if len(pend) > B_LAG:
                        stage_b(*pend.pop(0))
                for x in pend:
                    if Z_ON_PE and x[5] is None:
                        x = x[:5] + (stage_z(x[2]),)
                    stage_b(*x)
                while pending_s:
                    emit_s(pending_s.pop(0)[1])

            # ---- routing (b-linearity: logits(it) = b0 + u.(V0+..+V_{it-1}))
            s0_scale = 1.0 / J if not nonzero_b0 else 1.0
            if N_PASSES == 0:
                vfin = squash(s0, f32, sq_pool, s0_scale)
                s0_stack.close()
            else:
                v0 = squash(s0, fp16, sq_pool, s0_scale)
                s0_stack.close()
                open_routing_psum()
                pe_keepalive()
                vr0 = make_vrep(v0)
                s1 = routing_psum["s"].tile([BS, F], f32, tag="spsum")
                routing_pass(vr0, 1, s1)
                pe_keepalive()
                if N_PASSES == 1:
                    vfin = squash(s1, f32, sq_pool, 1.0)
                else:
                    v1 = squash(s1, fp16, sq_pool, 1.0)
                    vr1 = make_vrep(v1)
                    vr01 = vrep_pool.tile([128, F], fp16, tag="vr01")
                    nc.vector.tensor_tensor(vr01[:], vr0[:], vr1[:], op=add)
                    s2_ps = routing_psum["s"].tile([BS, F], f32, tag="spsum")
                    routing_pass(vr01, 2, s2_ps)
                    vfin = squash(s2_ps, f32, sq_pool, 1.0)
            nc.sync.dma_start(v_out, vfin[:])

    nc.compile()
    return nc


def _prep_inputs(inputs, W, b0, n_groups):
    """Host-side data layout. Returns (in_maps, nonzero_b0)."""
    g_ = n_groups
    i_ = g_ * IL
    nonzero_b0 = bool(np.any(b0[:i_]))

    wnp = ml_dtypes.float8_e4m3 if FP8_PH1 else np.float16

    def _pad(arr):
        """append one zero group so DoubleRow pair addressing stays in range"""
        if not FP8_PH1:
            return arr
        return np.concatenate([arr, np.zeros_like(arr[:1])], axis=0)

    w = np.ascontiguousarray(W[:i_]).astype(np.float32)
    # [i, j, d, e] -> [g, il, d, e, j] -> [g, 128, 512]
    wp = _pad(
        w.reshape(g_, IL, J, D, E)
        .transpose(0, 1, 3, 4, 2)
        .reshape(g_, 128, J * E)
        .astype(wnp)
    )

    shared = {"wp": wp}
    if nonzero_b0:
        c0 = b0[:i_].astype(np.float64)
        c0 = np.exp(c0 - c0.max(axis=1, keepdims=True))
        c0 = (c0 / c0.sum(axis=1, keepdims=True)).astype(np.float32)  # [i, J]
        # S0 accumulates x @ (c0-folded W) directly (inv_scale=1)
        w0 = w.reshape(g_, IL, J, D, E) * c0.reshape(g_, IL, J, 1, 1)
        wp0 = _pad(
            w0.transpose(0, 1, 3, 4, 2).reshape(g_, 128, J * E).astype(wnp)
        )
        shared["wp0"] = wp0
        # logits b0 replicated per (il, b) partition, fp16 for the PE add
        b0p = np.broadcast_to(
            b0[:i_].reshape(g_, IL, 1, J), (g_, IL, BS, J)
        )  # [g, il, b, j] ; partition = il*8+b
        shared["b0p"] = (
            np.ascontiguousarray(b0p.transpose(1, 2, 0, 3))
            .reshape(128, g_ * J)
            .astype(np.float16)
        )

    eye = np.eye(BS, dtype=np.float16)
    shared["ind"] = np.tile(eye, (IL, 1))          # [128, 8]
    shared["vind"] = shared["ind"].T.copy()        # [8, 128]
    shared["idn"] = np.eye(128, dtype=np.float16)  # [128, 128]
    shared["idb"] = np.eye(128, dtype=np.float32).astype(ml_dtypes.bfloat16)

    shared["msk"] = np.kron(
        np.eye(IL, dtype=np.float16), np.ones((D, BS), np.float16)
    )  # [128, 128], 1 where il == il2

    in_maps = []
    for c in range(NC_CORES):
        xc = inputs[c * BS:(c + 1) * BS, :i_].astype(np.float32)  # [8, i, d]
        xt = xc.reshape(BS, g_, IL, D).transpose(1, 2, 3, 0)      # [g, il, d, b]
        xsm = (
            np.ascontiguousarray(xt.transpose(1, 2, 0, 3)).reshape(128, g_ * BS)
        ).astype(wnp).reshape(128, g_, BS)
        in_maps.append(dict(shared, xs=xsm))
    return in_maps, nonzero_b0


def _get_program(n_groups, nonzero_b0):
    key = (n_groups, nonzero_b0)
    if key not in _CACHE:
        _CACHE[key] = _build_program(n_groups, nonzero_b0)
    return _CACHE[key]


def run_on_hw(inputs, W, b0, n_groups=G, trace=False):
    from concourse.bass_utils import run_bass_kernel_spmd

    in_maps, nonzero_b0 = _prep_inputs(inputs, W, b0, n_groups)
    nc = _get_program(n_groups, nonzero_b0)
    res = run_bass_kernel_spmd(nc, in_maps, list(range(NC_CORES)), trace=trace)
    outs = []
    for c in range(NC_CORES):
        v = res.results[c]["v_out"]                # [BS, 512] f32, (e,j) layout
        outs.append(v.reshape(BS, E, J).transpose(0, 2, 1))  # [BS, J, E]
    return np.concatenate(outs, axis=0).astype(np.float32), res


def kernel(inputs, W, b0):
    inputs = np.asarray(inputs, dtype=np.float32)
    W = np.asarray(W, dtype=np.float32)
    b0 = np.asarray(b0, dtype=np.float32)
    out, _ = run_on_hw(inputs, W, b0)
    return out
